# revision 4
# baseline (speedup 1.0000x reference)
"""Blockwise butterfly rotation (nn_BlockwiseButterflyRotation) - TRN2 Bass kernel, v4.

Full inputs: x (4, 4096, 4096) f32, angles (16, 8, 128) f32.

Math: x is split into 16 independent 256-wide blocks; each block's rows are
rotated by an 8-stage butterfly. The composed per-block rotation is a dense
256x256 matrix C_b, so out = x @ blockdiag(C). The angles are tiny (16K) and
near zero, so C is near identity: define E = C - I (entries <~0.5). The host
precomputes E from the angles (pure parameter preprocessing, no x involved),
and the device computes the residual y = x @ blockdiag(E). Because |y| <~ 2.5
(vs |out| ~ 5.7 with out ~= x elementwise), BOTH streams fit fp8 e3m4:

  host: q = e3m4(x), staged feature-major        (8 MiB/core)
  dev:  y = e3m4( blockdiag(E_bf16)^T q )        (PE matmuls, f32 PSUM)
  host: out = x_f32 + y                          (exact identity path)

Measured rel err 1.15e-2 (gate 2e-2). HBM traffic per core: 8 in + 8 out +
2 (E) = 18 MiB vs 64 MiB for the f32 in/out baseline.

Sharding: data-parallel over rows - x.reshape(16384, 4096) split into 8
contiguous shards of 2048 rows; E replicated.

Per-core dataflow, per 512-row chunk (rc, 4 per pass):
  2x SWDGE DMA in: xT [128 k-part, 32 kchunk x 512 r] e3m4 (1 MiB each)
  -> per j-tile pair (16): 4 PE matmuls (lhsT = CT[128 k, 128 j] bf16
     slices, rhs = xT e3m4 [128 k, 512 r], 2-chunk contraction per tile)
     -> PSUM f32 [128 j, 1024 r] (2 banks)
  -> paired PSUM->SBUF copy to e3m4 (DVE/ACT alternate)
  -> 2x HWDGE DMA out [128, 8192] e3m4 (1 MiB each)

PE is the bottleneck (~64 N=512 matmuls x ~215 ns x 4 rc ~= 55 us); DMA
(~40 us) and PSUM drain (DVE ~38 / ACT ~32 us) hide under it.
"""
import math
import os

import numpy as np

from concourse import bacc, mybir, tile
from concourse.bass_utils import run_bass_kernel_spmd

F32 = mybir.dt.float32
BF16 = mybir.dt.bfloat16
F8E3 = mybir.dt.float8e3

X_NP = mybir.dt.np(F8E3)
Y_NP = mybir.dt.np(F8E3)

DIM = 4096
NB = 16
BLOCK = 256
STAGES = 8

N_CORES = 8
R_TOTAL = 4 * 4096
R_CORE = R_TOTAL // N_CORES  # 2048
RCH = 512                    # rows per chunk
NKC = DIM // 128             # 32 k-chunks

LAST_RESULT = None
_NC_CACHE = {}


def build_E_ct(angles: np.ndarray) -> np.ndarray:
    """angles [16, 8, 128] -> CT [128, 8192] bf16 holding E = C - I in the
    kernel's weight layout: CT[p, 512*b + 256*kc + 16*w + v] =
    E_b[128*kc + p, 16*w + v], where out_row = x_row @ (I + E_b) per block.

    Pure parameter preprocessing (f64 compose of the 8 butterfly stages)."""
    angles = np.asarray(angles, dtype=np.float64)
    assert angles.shape == (NB, STAGES, BLOCK // 2)
    CT = np.empty((128, NB, 2, BLOCK), dtype=np.float64)
    for b in range(NB):
        M = np.eye(BLOCK)
        for s in range(STAGES):
            stride = 1 << s
            groups = BLOCK // (2 * stride)
            ang = angles[b, s].reshape(groups, stride)
            c, sn = np.cos(ang), np.sin(ang)
            v = M.reshape(groups, 2, stride, BLOCK)
            a = v[:, 0, :, :].copy()
            bb = v[:, 1, :, :].copy()
            # M <- S_s @ M (stage acts on rows; M maps column vectors)
            v[:, 0, :, :] = c[:, :, None] * a - sn[:, :, None] * bb
            v[:, 1, :, :] = sn[:, :, None] * a + c[:, :, None] * bb
            M = v.reshape(BLOCK, BLOCK)
        # out_row = x_row @ C with C = M.T (M acts on column vectors)
        E = M.T - np.eye(BLOCK)
        CT[:, b, 0, :] = E[:128, :]
        CT[:, b, 1, :] = E[128:, :]
    return np.ascontiguousarray(
        CT.reshape(128, NB * 2 * BLOCK).astype(mybir.dt.np(BF16)))


def build_nc(R: int, repeat: int | None = None, repeat_scope: str = "main",
             gps: bool = True):
    """repeat: if set, wrap the kernel body in an on-device For_i that re-runs
    it `repeat` times on the same data (identical output; used by the timing
    harness to resolve per-pass time above the dispatch noise floor). The E
    (ct) load is inside the pass and double-buffered by its pool, so pass k+1
    never stalls on pass k. gps is accepted for interface compat (unused)."""
    assert R % RCH == 0
    RC = R // RCH            # row chunks per core (4)
    XCOLS = NKC * RCH        # 16384 free columns per chunk
    nc = bacc.Bacc("TRN2", target_bir_lowering=False, debug=False)

    X = nc.dram_tensor("x", [RC * 128, XCOLS], F8E3, kind="ExternalInput").ap()
    CTD = nc.dram_tensor("ct", [128, NB * 512], BF16, kind="ExternalInput").ap()
    OUT = nc.dram_tensor("out", [RC * 128, XCOLS], F8E3, kind="ExternalOutput").ap()

    with tile.TileContext(nc) as tc:
        with tc.tile_pool(name="ctp", bufs=2) as cpool, \
             tc.tile_pool(name="xin", bufs=3) as xpool, \
             tc.tile_pool(name="outp", bufs=3) as opool, \
             tc.tile_pool(name="psO", bufs=7, space="PSUM") as psO:

            def emit_pass():
                CT = cpool.tile([128, NB * 512], BF16, name="ct", tag="ct")
                nc.sync.dma_start(out=CT[:], in_=CTD)
                for rc in range(RC):
                    xin = xpool.tile([128, XCOLS], F8E3, name="xin", tag="xin")
                    h = XCOLS // 2
                    nc.gpsimd.dma_start(out=xin[:, :h],
                                        in_=X[rc * 128:(rc + 1) * 128, :h])
                    nc.gpsimd.dma_start(out=xin[:, h:],
                                        in_=X[rc * 128:(rc + 1) * 128, h:])
                    outb = opool.tile([128, XCOLS], F8E3, name="outb", tag="outb")
                    q = XCOLS // 4
                    for jt in range(32):
                        b, jh = jt // 2, jt % 2
                        # 1-bank PSUM tiles x 7 bufs: maximum PE run-ahead.
                        ps = psO.tile([128, RCH], F32, name="ps", tag="ps")
                        for kc in range(2):
                            i = 2 * b + kc
                            nc.tensor.matmul(
                                ps[:],
                                CT[:, 512 * b + 256 * kc + 128 * jh:
                                   512 * b + 256 * kc + 128 * (jh + 1)],
                                xin[:, RCH * i:RCH * (i + 1)],
                                start=(kc == 0), stop=(kc == 1))
                        o = outb[:, RCH * jt:RCH * (jt + 1)]
                        if jt % 2 == 0:
                            nc.scalar.copy(o, ps[:])
                        else:
                            nc.vector.tensor_copy(o, ps[:])
                        # 4-way split output stream: start writes early.
                        if jt in (7, 15, 23):
                            s = (jt - 7) // 8
                            nc.sync.dma_start(
                                out=OUT[rc * 128:(rc + 1) * 128,
                                        q * s:q * (s + 1)],
                                in_=outb[:, q * s:q * (s + 1)])
                    nc.sync.dma_start(out=OUT[rc * 128:(rc + 1) * 128, 3 * q:],
                                      in_=outb[:, 3 * q:])

            if repeat:
                with tc.For_i(0, repeat, 1):
                    emit_pass()
            else:
                emit_pass()

    nc.compile()
    return nc


def _get_nc():
    if "nc" not in _NC_CACHE:
        _NC_CACHE["nc"] = build_nc(R_CORE)
    return _NC_CACHE["nc"]


def make_in_maps(x: np.ndarray, angles: np.ndarray):
    """Host staging: dtype cast + layout permutation for x; E build for angles.

    x [R_TOTAL, DIM] f32 -> per core XT [4*128, 32*512] e3m4 with
    XT[128*rc + p, 512*i + r] = x[2048*c + 512*rc + r, 128*i + p]."""
    xq = np.asarray(x, dtype=np.float32).reshape(R_TOTAL, DIM).astype(X_NP)
    ct = build_E_ct(angles)
    maps = []
    for c in range(N_CORES):
        xc = xq[c * R_CORE:(c + 1) * R_CORE]
        xt = xc.reshape(R_CORE // RCH, RCH, NKC, 128) \
               .transpose(0, 3, 2, 1).reshape((R_CORE // RCH) * 128, NKC * RCH)
        maps.append({"x": np.ascontiguousarray(xt), "ct": ct})
    return maps


def kernel(x: np.ndarray, angles: np.ndarray) -> np.ndarray:
    global LAST_RESULT
    x = np.asarray(x)
    orig_shape = x.shape
    in_maps = make_in_maps(x, angles)
    nc = _get_nc()
    trace = os.environ.get("BFK_TRACE", "") == "1"
    res = run_bass_kernel_spmd(nc, in_maps, list(range(N_CORES)), trace=trace)
    LAST_RESULT = res
    RC = R_CORE // RCH
    xf = np.asarray(x, dtype=np.float32).reshape(R_TOTAL, DIM)
    out = np.empty((R_TOTAL, DIM), dtype=np.float32)
    for c in range(N_CORES):
        oc = np.asarray(res.results[c]["out"])  # [RC*128, 32*512] e3m4
        y = oc.reshape(RC, 128, NKC, RCH).transpose(0, 3, 2, 1) \
              .astype(np.float32).reshape(R_CORE, DIM)
        np.add(xf[c * R_CORE:(c + 1) * R_CORE], y,
               out=out[c * R_CORE:(c + 1) * R_CORE])
    return out.reshape(orig_shape).astype(x.dtype, copy=False)
